# revision 1
# baseline (speedup 1.0000x reference)
"""Trainium2 Bass kernel for DynamicFilterWithImageInput (v2: hybrid
mean-filter banded + fp8 DoubleRow residual).

Model (per batch b):
  img_feat = mean_hw(relu(BN1(conv2d(raw_img, w_conv1, 3x3, zeropad=1) + b1)))   # (64,)
  w        = softmax_over_C(BN2(img_feat @ w_filt.T + b_filt).reshape(C, K*K))   # (C, 25)
  out      = depthwise_conv5x5(reflect_pad(x_feat), w)                           # (C, H, W)

Decomposition: w[c,t] = wbar[t] + dw[c,t] with wbar = mean over channels.
  out = (wbar-conv of x)  +  (dw depthwise conv of x)
  - wbar part: channel-shared 5x5 conv -> banded matmuls over the row axis
    in a transposed layout [row, (x, c)].  5 vertical taps per PE
    column-cycle; vertical reflection folded into the 128x128 band
    matrices A_j = sum_i wbar[i,j]*M_i (masks M_i host-built, pre-scaled
    by 1/C so the on-device channel-sum of w serves as wbar).
  - dw residual (|dw| ~ 15% of w): fp8(e4m3) diagonal matmuls in
    DoubleRow perf mode -> 2 taps per pass at 0.5 cycles/row.  dw scaled
    x256 into fp8 range; 1/256 folded into the PSUM evacuation.
  Host adds the two partial outputs (they are produced in different
  layouts).

Sharding: pure data-parallel over batch (16 batches -> 8 cores x 2).
"""

import sys

sys.path.insert(0, "/opt/trn_rl_repo")

import numpy as np
import ml_dtypes

import concourse.bass as bass
import concourse.bacc as bacc
import concourse.mybir as mybir
import concourse.tile as tile
from concourse.bass_utils import run_bass_kernel_spmd
import concourse.dve_ops as _dve_ops


def _get_pair_mac():
    """Fused custom DVE op: out = in0*s0 + in1*s1 (two conv taps per pass)."""
    if hasattr(_dve_ops, "PAIR_MAC_ANT"):
        return _dve_ops.PAIR_MAC_ANT
    from concourse.dve_spec import Spec, Src0, Src1, C0, C1
    op = _dve_ops.DveOp(
        "PAIR_MAC_ANT",
        Spec(
            body=Src0 * C0 + Src1 * C1,
            reference=lambda in0, in1, s0, s1, imm2: (
                in0.astype(np.float32) * s0 + in1.astype(np.float32) * s1
            ).astype(np.float32),
        ),
        subdim=False,
        uops_sha={"v3": "f2ac165a27dbafb3", "v4": "49eb47656a95aba3"},
    )
    _dve_ops.OPS.append(op)
    _dve_ops.CUSTOM_DVE_SPECS[op.name] = op.spec
    _dve_ops._SUB_OPCODE_FOR_NAME[op.name] = (
        _dve_ops._CUSTOM_DVE_ROW_BASE + len(_dve_ops.OPS) - 1
    )
    assert max(_dve_ops._SUB_OPCODE_FOR_NAME.values()) < 0x20
    _dve_ops.PAIR_MAC_ANT = op
    return op


PAIR_MAC_ANT = _get_pair_mac()

F8NP = ml_dtypes.float8_e4m3

F8 = mybir.dt.float8e4
F16 = mybir.dt.float16
F32 = mybir.dt.float32
AF = mybir.ActivationFunctionType
ALU = mybir.AluOpType
DR = mybir.MatmulPerfMode.DoubleRow

EPS = 1e-5
B_PC = 2          # batches per core
C = 256           # channels
CG = C // 128     # channel groups of 128
K5 = 5
TAPS = [(i, j) for i in range(K5) for j in range(K5)]
NSLAB = B_PC * CG
DW_SCALE = 256.0  # residual filter scale into fp8 range

_PROG_CACHE = {}


def _as_strided(ap, dims, offset=None):
    n = ap.copy()
    v = n.ap
    v.clear()
    v.extend([list(d) for d in dims])
    if offset is not None:
        n.offset = offset
    return n


def _build_program(H, W):
    Hp, Wp = H + 4, W + 4
    HWOUT = H * W
    QR = 8                        # output rows per residual quad
    GR = 4                        # rows per matmul group
    NQ = H // QR                  # quads per slab
    N1CH = 512                    # conv1 psum chunk
    N1 = HWOUT // N1CH
    IMCH = min(4 * N1CH, HWOUT)   # im2col streaming chunk
    XG = 8                        # x-cols per rank psum group
    NXG = W // XG                 # 16 groups
    XPM = 4                       # x-cols per rank matmul (N=512)

    nc = bacc.Bacc("TRN2", target_bir_lowering=False, debug=False)

    x8_d = nc.dram_tensor("x8", [NSLAB, 128, Hp * Wp], F8, kind="ExternalInput").ap()
    xt_d = nc.dram_tensor("xt", [NSLAB, 128, Wp * 128], F16, kind="ExternalInput").ap()
    im2col_d = nc.dram_tensor("im2col", [54, HWOUT], F16, kind="ExternalInput").ap()
    wconv_d = nc.dram_tensor("wconv", [54, 128], F16, kind="ExternalInput").ap()
    b1r_d = nc.dram_tensor("b1r", [128, 1], F32, kind="ExternalInput").ap()
    wft_d = nc.dram_tensor("wft", [65, C, 25], F16, kind="ExternalInput").ap()
    ident_d = nc.dram_tensor("ident", [128, 128], F16, kind="ExternalInput").ap()
    id32_d = nc.dram_tensor("id32", [128, 128], F32, kind="ExternalInput").ap()
    masks_d = nc.dram_tensor("masks", [128, K5 * 128], F16, kind="ExternalInput").ap()
    out_d = nc.dram_tensor("out", [B_PC, C, H, W], F16, kind="ExternalOutput").ap()
    y_d = nc.dram_tensor("yrk", [NSLAB, 128, W, 128], F16, kind="ExternalOutput").ap()

    # dram scratch
    imgf_d = nc.dram_tensor("imgf_sc", [128], F32).ap()
    df_d = nc.dram_tensor("df_sc", [B_PC, 25, C], F32).ap()
    wbar_d = nc.dram_tensor("wbar_sc", [B_PC * 25], F32).ap()

    # residual split: 22 taps on PE (11 DoubleRow pairs), 3 on DVE
    PE_PAIRS = [(TAPS[2 * k], TAPS[2 * k + 1]) for k in range(11)]
    DVE_PAIRS = [(TAPS[22], TAPS[23]), (TAPS[24], None)]

    with tile.TileContext(nc) as tc:
        with (
            tc.tile_pool(name="consts", bufs=1) as consts,
            tc.tile_pool(name="p0", bufs=1) as p0,
            tc.tile_pool(name="imc", bufs=2) as imcp,
            tc.tile_pool(name="trash", bufs=2) as trashp,
            tc.tile_pool(name="x8p", bufs=3) as x8p,
            tc.tile_pool(name="xtp", bufs=2) as xtp,
            tc.tile_pool(name="diag", bufs=len(PE_PAIRS) * NSLAB) as diagp,
            tc.tile_pool(name="q", bufs=4) as qp,
            tc.tile_pool(name="Ap", bufs=1) as Apool,
            tc.tile_pool(name="ot", bufs=4) as otp,
            tc.tile_pool(name="ot2", bufs=6) as ot2p,
            tc.tile_pool(name="psum", bufs=2, space="PSUM") as psump,
        ):
            # ---------- constants ----------
            wconv = consts.tile([54, 128], F16)
            b1r = consts.tile([128, 1], F32)
            wft = consts.tile([65, C, 25], F16)
            ident = consts.tile([128, 128], F16)
            id32 = consts.tile([128, 128], F32)
            masks = consts.tile([128, K5, 128], F16)
            zeros = consts.tile([128, N1CH], F32)
            nc.sync.dma_start(wconv[:], wconv_d[:])
            nc.sync.dma_start(b1r[:], b1r_d[:])
            nc.sync.dma_start(wft[:], wft_d[:])
            nc.sync.dma_start(ident[:], ident_d[:])
            nc.sync.dma_start(id32[:], id32_d[:])
            nc.sync.dma_start(
                masks[:], masks_d[:].rearrange("p (a b) -> p a b", a=K5, b=128))
            nc.gpsimd.memset(zeros[:], 0.0)

            # x8 slab loads (double buffered)
            x8s = [None] * NSLAB

            # input loads ride the ACT hwdge queue so they never queue
            # behind the (big) output writes on the SP queue
            def load_x8(s):
                # +8 spare tail so DVE full-width flat runs stay in bounds
                t = x8p.tile([128, Hp * Wp + 8], F8, tag="x8")
                nc.vector.memset(t[:, Hp * Wp:], 0.0)
                nc.sync.dma_start(t[:, 0:Hp * Wp], x8_d[s])
                x8s[s] = t

            # xT quarter loads
            xts = [None] * NSLAB

            def load_xt(s):
                t = xtp.tile([128, Wp, 128], F16, tag="xt")
                nc.sync.dma_start(
                    t[:], xt_d[s].rearrange("p (a b) -> p a b", a=Wp, b=128))
                xts[s] = t

            # PE warmup: ramp the tensor engine to full p-state while the
            # input DMAs stream (results unused)
            for wi in range(24):
                psw = psump.tile([128, 512], F32, tag="ps")
                nc.tensor.matmul(
                    psw[:], ident[:], masks[:, 0:4, :], start=True, stop=True)

            # ---------- conv1: streamed im2col, relu+sum split ACT/DVE ------
            acc = p0.tile([128, N1], F32)
            imt = None
            for ci in range(N1):
                if ci % (IMCH // N1CH) == 0:
                    imt = imcp.tile([54, IMCH], F16, tag="imc")
                    o0 = ci * N1CH
                    nc.sync.dma_start(imt[:], im2col_d[:, o0:o0 + IMCH])
                ps1 = psump.tile([128, N1CH], F32, tag="ps")
                off = (ci % (IMCH // N1CH)) * N1CH
                nc.tensor.matmul(
                    ps1[:], wconv[:], imt[:, off:off + N1CH],
                    start=True, stop=True,
                )
                tr = trashp.tile([128, N1CH], F32, tag="tr")
                if ci % 2 == 0:
                    nc.scalar.activation(
                        tr[:], ps1[:], AF.Relu, bias=b1r[:], scale=1.0,
                        accum_out=acc[:, ci:ci + 1],
                    )
                else:
                    nc.vector.scalar_tensor_tensor(
                        tr[:], ps1[:], b1r[:], zeros[:], ALU.add, ALU.max,
                        accum_out=acc[:, ci:ci + 1],
                    )
            load_x8(0)
            load_x8(1)
            load_xt(0)
            sfeat = p0.tile([128, 1], F32)
            nc.vector.tensor_reduce(sfeat[:], acc[:], mybir.AxisListType.X, ALU.add)
            nc.sync.dma_start(imgf_d[:], sfeat[:])

            imgfT32 = p0.tile([65, B_PC], F32)
            nc.sync.dma_start(
                imgfT32[0:64, :],
                imgf_d[:].rearrange("(b o) -> o b", b=B_PC, o=64),
            )
            imgfT = p0.tile([65, B_PC], F16)
            nc.vector.memset(imgfT[64:65, :], 1.0)
            nc.scalar.mul(imgfT[0:64, :], imgfT32[0:64, :], 1.0 / HWOUT)

            # dense: df[b, t, c] (+bias row)
            CH_PER_SLOT = 2
            t0 = 0
            while t0 < 25:
                tws = []
                t1 = t0
                while t1 < 25 and len(tws) < CH_PER_SLOT:
                    tw = min(2, 25 - t1)
                    tws.append((t1, tw))
                    t1 += tw
                tot = t1 - t0
                psd = psump.tile([B_PC, tot, C], F32, tag="ps")
                for (tt, tw) in tws:
                    nc.tensor.matmul(
                        psd[:, tt - t0:tt - t0 + tw, :], imgfT[:],
                        wft[:, :, tt:tt + tw].transpose([0, 2, 1]),
                        start=True, stop=True,
                    )
                dfc = trashp.tile([B_PC, tot, C], F32, tag="dfc")
                nc.scalar.copy(dfc[:], psd[:])
                nc.sync.dma_start(df_d[:, t0:t0 + tot, :], dfc[:])
                t0 = t1

            # softmax over channels at partitions b*32 .. b*32+25
            dfsb = p0.tile([B_PC * 32, C], F32)
            edf = p0.tile([B_PC * 32, C], F32)
            ssum = p0.tile([B_PC * 32, 1], F32)
            rsum = p0.tile([B_PC * 32, 1], F32)
            wsm = p0.tile([B_PC * 32, C], F32)
            wbars = p0.tile([B_PC * 32, 1], F32)
            wbarm = p0.tile([B_PC * 32, 1], F32)
            dws = p0.tile([B_PC * 32, C], F32)
            for b in range(B_PC):
                sl = slice(b * 32, b * 32 + 25)
                nc.sync.dma_start(dfsb[sl, :], df_d[b])
                nc.scalar.activation(edf[sl, :], dfsb[sl, :], AF.Exp)
                nc.vector.tensor_reduce(
                    ssum[sl, :], edf[sl, :], mybir.AxisListType.X, ALU.add)
                nc.vector.reciprocal(rsum[sl, :], ssum[sl, :])
                nc.vector.tensor_scalar(
                    wsm[sl, :], edf[sl, :], rsum[sl, :], None, ALU.mult)
                # channel-sum of w (-> wbar via host-scaled masks), mean, dw
                nc.vector.tensor_reduce(
                    wbars[sl, :], wsm[sl, :], mybir.AxisListType.X, ALU.add)
                nc.scalar.mul(wbarm[sl, :], wbars[sl, :], 1.0 / C)
                nc.vector.tensor_scalar(
                    dws[sl, :], wsm[sl, :], wbarm[sl, :], None, ALU.subtract)
                nc.sync.dma_start(wbar_d[b * 25:(b + 1) * 25], wbars[sl, :])

            # wbar replicated on all partitions [128, B_PC*25]
            wbrep = p0.tile([128, B_PC * 25], F32)
            nc.sync.dma_start(
                wbrep[:], _as_strided(wbar_d[:], [[0, 128], [1, B_PC * 25]], 0))

            # band matrices A[b][j] = sum_i wbar[b,i,j] * M_i (masks carry 1/C)
            Amats = []
            for b in range(B_PC):
                Aj = []
                for j in range(K5):
                    A = Apool.tile([128, 128], F16, tag=f"A{b}{j}")
                    sc0 = wbrep[:, b * 25 + 0 * K5 + j: b * 25 + 0 * K5 + j + 1]
                    nc.vector.tensor_scalar(A[:], masks[:, 0, :], sc0, None, ALU.mult)
                    for i in range(1, K5):
                        sci = wbrep[:, b * 25 + i * K5 + j: b * 25 + i * K5 + j + 1]
                        nc.vector.scalar_tensor_tensor(
                            A[:], masks[:, i, :], sci, A[:], ALU.mult, ALU.add)
                    Aj.append(A)
                Amats.append(Aj)

            # per-slab residual filters [128(c), 25] via PE transpose, x256
            vts = []
            for s in range(NSLAB):
                b, cg = divmod(s, CG)
                pst = psump.tile([128, 25], F32, tag="ps")
                nc.tensor.transpose(
                    pst[:], dws[b * 32:b * 32 + 25, cg * 128:(cg + 1) * 128],
                    id32[b * 32:b * 32 + 25, 0:25],
                )
                vt = p0.tile([128, 25], F32, tag=f"vt{s}")
                nc.scalar.mul(vt[:], pst[:], DW_SCALE)
                vts.append(vt)

            # fp8 diag pair tiles (PE taps only)
            dts_all = []
            for s in range(NSLAB):
                dts = []
                for k, (ta, tb) in enumerate(PE_PAIRS):
                    dtp = diagp.tile([128, 2, 128], F8, tag="dt")
                    ia = ta[0] * K5 + ta[1]
                    nc.scalar.mul(dtp[:, 0, :], ident[:], vts[s][:, ia:ia + 1])
                    ib = tb[0] * K5 + tb[1]
                    nc.scalar.mul(dtp[:, 1, :], ident[:], vts[s][:, ib:ib + 1])
                    dts.append(dtp)
                dts_all.append(dts)

            def rank_group(s, xg):
                b, cg = divmod(s, CG)
                xt = xts[s]
                x0 = xg * XG
                ps = psump.tile([128, XG, 128], F32, tag="ps2")
                ot2 = ot2p.tile([128, XG, 128], F16, tag="ot2")
                for j in range(K5):
                    for xm in range(XG // XPM):
                        rhs = _as_strided(
                            xt[:],
                            [[Wp * 128, 128], [128, XPM], [1, 128]],
                            (x0 + xm * XPM + j) * 128,
                        )
                        nc.tensor.matmul(
                            ps[:, xm * XPM:(xm + 1) * XPM, :],
                            Amats[b][j][:], rhs,
                            start=(j == 0), stop=(j == K5 - 1),
                        )
                nc.vector.tensor_copy(ot2[:], ps[:])
                nc.sync.dma_start(y_d[s][:, x0:x0 + XG, :], ot2[:])

            # fill the softmax/decomp-era PE idle with early rank groups
            RANK_PRE = 6
            for _xg in range(RANK_PRE):
                rank_group(0, _xg)

            # ---------- residual depthwise (fp8 DoubleRow) ----------
            for s in range(NSLAB):
                b, cg = divmod(s, CG)
                x8 = x8s[s]
                dts = dts_all[s]
                if s + 2 < NSLAB:
                    load_x8(s + 2)
                for q in range(NQ):
                    y0 = q * QR
                    ps = psump.tile([128, QR, W], F32, tag="ps")
                    ot = otp.tile([128, QR, W], F16, tag="ot")
                    XFREE = Hp * Wp + 8
                    for k, (ta, tb) in enumerate(PE_PAIRS):
                        ia, ja = ta
                        offa = (y0 + ia) * Wp + ja
                        delta = (tb[0] - ta[0]) * Wp + (tb[1] - ta[1])
                        for g in range(QR // GR):
                            rhs = _as_strided(
                                x8[:],
                                [[XFREE, 128], [delta, 2], [Wp, GR], [1, W]],
                                offa + g * GR * Wp,
                            )
                            nc.tensor.matmul(
                                ps[:, g * GR:(g + 1) * GR, :],
                                dts[k][:], rhs,
                                start=(k == 0), stop=(k == len(PE_PAIRS) - 1),
                                perf_mode=DR,
                            )

                    # DVE taps: contiguous full-padded-width runs over x8
                    def run(i, j):
                        off = (y0 + i) * Wp + j
                        return x8[:, off:off + QR * Wp]

                    def sc(i, j):
                        t = i * K5 + j
                        return vts[s][:, t:t + 1]

                    # 5 DVE taps: q1=(19,20), q2=(21,22), q3=tap23; sum
                    qs = []
                    for (ta, tb) in DVE_PAIRS:
                        qt = qp.tile([128, QR * Wp], F16, tag="q")
                        nc.vector._custom_dve(
                            PAIR_MAC_ANT, out=qt[:],
                            in0=run(*ta), in1=run(*(tb or ta)),
                            s0=sc(*ta), s1=(sc(*tb) if tb else 0.0))
                        qs.append(qt)
                    for qtile in qs[1:]:
                        nc.vector.tensor_tensor(
                            qs[0][:], qs[0][:], qtile[:], ALU.add)
                    qv = qs[0][:].rearrange(
                        "p (a b) -> p a b", a=QR, b=Wp)[:, :, 0:W]
                    # ot = qv + ps  (both carry x256; host divides)
                    nc.vector.scalar_tensor_tensor(
                        ot[:], qv, 1.0, ps[:], ALU.mult, ALU.add)
                    nc.sync.dma_start(
                        out_d[b, cg * 128:(cg + 1) * 128, y0:y0 + QR, :], ot[:])



            # ---------- rank (wbar) banded pass (remainder) ----------
            for s in range(NSLAB):
                if s + 1 < NSLAB:
                    load_xt(s + 1)
                for xg in range(RANK_PRE if s == 0 else 0, NXG):
                    rank_group(s, xg)

    nc.compile()
    return nc


def get_program(H, W):
    key = (H, W)
    if key not in _PROG_CACHE:
        _PROG_CACHE[key] = _build_program(H, W)
    return _PROG_CACHE[key]


def _reflect_idx(r, n):
    if r < 0:
        return -r
    if r >= n:
        return 2 * n - 2 - r
    return r


def host_prep(x_feat, raw_img, w_conv1, b_conv1, g1, beta1, m1, v1,
              w_filt, b_filt, g2, beta2, m2, v2):
    B, Cc, H, W = x_feat.shape
    assert Cc == C
    n_cores = B // B_PC
    Hp, Wp = H + 4, W + 4

    a1 = g1 / np.sqrt(v1 + EPS)
    w1f = (w_conv1 * a1[:, None, None, None]).astype(np.float32)
    b1f = (b_conv1 - m1) * a1 + beta1

    a2 = g2 / np.sqrt(v2 + EPS)
    wff = (w_filt * a2[:, None]).astype(np.float32)
    bff = (b_filt - m2) * a2 + beta2

    wft = np.empty((65, C, 25), np.float32)
    wft[:64] = wff.reshape(C, 25, 64).transpose(2, 0, 1)
    wft[64] = bff.reshape(C, 25)
    wft16 = wft.astype(np.float16)

    b1r = np.tile(b1f, B_PC).reshape(128, 1).astype(np.float32)

    ident = np.eye(128, dtype=np.float16)
    id32 = np.zeros((128, 128), np.float32)
    for b in range(B_PC):
        id32[b * 32:b * 32 + 25, 0:25] = np.eye(25)

    # vertical-reflection masks, pre-scaled by 1/C (channel-sum -> mean)
    masks = np.zeros((128, K5, 128), np.float32)
    for i in range(K5):
        for yout in range(H):
            yin = _reflect_idx(yout + i - 2, H)
            masks[yin, i, yout] = 1.0 / C
    masks16 = masks.reshape(128, K5 * 128).astype(np.float16)

    # x: reflect pad
    xpad16 = np.pad(x_feat, ((0, 0), (0, 0), (2, 2), (2, 2)),
                    mode="reflect").astype(np.float16)
    x8 = xpad16.astype(F8NP).reshape(B, CG, 128, Hp * Wp)

    # transposed (cols-only padded) layout per slab: [y, x, c]
    xpc = np.pad(x_feat, ((0, 0), (0, 0), (0, 0), (2, 2)),
                 mode="reflect").astype(np.float16)          # (B, C, H, Wp)
    xt = np.ascontiguousarray(
        xpc.reshape(B, CG, 128, H, Wp).transpose(0, 1, 3, 4, 2)
    ).reshape(B, CG, 128, Wp * 128)                          # [b, cg, y, (x c)]

    # conv1 im2col
    rawpad = np.pad(raw_img, ((0, 0), (0, 0), (1, 1), (1, 1))).astype(np.float32)
    wconv = np.zeros((54, 128), np.float32)
    w_flat = w1f.transpose(1, 2, 3, 0).reshape(27, 64)
    for b in range(B_PC):
        wconv[b * 27:(b + 1) * 27, b * 64:(b + 1) * 64] = w_flat
    wconv16 = wconv.astype(np.float16)

    in_maps = []
    for core in range(n_cores):
        bs = core * B_PC
        im2col = np.empty((54, H * W), np.float32)
        for b in range(B_PC):
            for c in range(3):
                for i in range(3):
                    for j in range(3):
                        p = b * 27 + c * 9 + i * 3 + j
                        im2col[p] = rawpad[bs + b, c, i:i + H, j:j + W].reshape(-1)
        in_maps.append({
            "x8": np.ascontiguousarray(x8[bs:bs + B_PC]).reshape(
                NSLAB, 128, Hp * Wp),
            "xt": np.ascontiguousarray(xt[bs:bs + B_PC]).reshape(
                NSLAB, 128, Wp * 128),
            "im2col": im2col.astype(np.float16),
            "wconv": wconv16,
            "b1r": b1r,
            "wft": wft16,
            "ident": ident,
            "id32": id32,
            "masks": masks16,
        })
    return in_maps


def run(inputs, trace=False, **_ignored):
    x_feat = inputs["x_feat"]
    B, _, H, W = x_feat.shape
    nc = get_program(H, W)
    in_maps = host_prep(**inputs)
    n_cores = len(in_maps)
    res = run_bass_kernel_spmd(nc, in_maps, list(range(n_cores)), trace=trace)
    outs = []
    for r in res.results:
        resid = r["out"].astype(np.float32) * (1.0 / DW_SCALE)   # (B_PC, C, H, W)
        yr = r["yrk"].astype(np.float32)               # (NSLAB, 128, W, 128)
        rank = yr.reshape(B_PC, CG, 128, W, 128).transpose(0, 1, 4, 2, 3)
        rank = rank.reshape(B_PC, C, H, W)
        outs.append(resid + rank)
    out = np.concatenate(outs, axis=0)
    return out, res


def kernel(**inputs) -> np.ndarray:
    out, _ = run(inputs, trace=False)
    return out

